# revision 1
# baseline (speedup 1.0000x reference)
"""Longformer sliding-chunk attention (B=2, S=4096, E=1024, H=16, W=256) on 8 trn2 cores.

Sharding: tensor-parallel over heads — core c owns heads {2c, 2c+1}. Each core:
  - projects q/k/v for its 128 output features (2 heads x 64) over the full
    [8192, 1024] hidden states, directly in transposed [d, s] layout
  - computes chunked attention fully transposed: scoresT = K @ Q^T per
    128-key-block, exp on ACT (no max subtraction: scores are O(1) for this
    problem), probsT @ V via PE with an appended ones-column that yields the
    softmax denominators for free
  - ships unnormalized numerator^T [128, 8192] + denominators [2, 8192]
Host adds the boundary-mask pad mass to denominators and normalizes.

All matmuls run in float32r (full-rate fp32 mode, ~1e-4 rounding).
"""
import numpy as np

import concourse.bass as bass
import concourse.mybir as mybir
import concourse.tile as tile
from concourse import bacc
from concourse.bass_utils import run_bass_kernel_spmd
from concourse.masks import make_identity

F32 = mybir.dt.float32
F32R = mybir.dt.float32r
AFT = mybir.ActivationFunctionType

B, S, E = 2, 4096, 1024
H, W, D = 16, 256, 64
BS = B * S           # 8192
NT = 16              # 512-wide seq tiles over BS for projections
KT = 8               # contraction tiles of 128 over E
NCHUNK = S // W      # 16 chunks per batch
NKB = S // 128       # 32 key blocks of 128 per batch

_NC_CACHE = None


def _build():
    nc = bacc.Bacc("TRN2", target_bir_lowering=False, debug=False, num_devices=8)

    hsT = nc.dram_tensor("hsT", [E, BS], F32R, kind="ExternalInput").ap()
    w_ap = {}
    b_ap = {}
    for nm in ("q", "k", "v"):
        w_ap[nm] = nc.dram_tensor(f"w{nm}T", [E, 128], F32R, kind="ExternalInput").ap()
        b_ap[nm] = nc.dram_tensor(f"b{nm}", [128, 1], F32, kind="ExternalInput").ap()
    ones2 = nc.dram_tensor("ones2", [128, 24], F32R, kind="ExternalInput").ap()
    outT = nc.dram_tensor("outT", [130, BS], F32, kind="ExternalOutput").ap()

    with tile.TileContext(nc) as tc:
        with (
            tc.tile_pool(name="singles", bufs=1) as singles,
            tc.tile_pool(name="big", bufs=1) as big,
            tc.tile_pool(name="hst", bufs=4) as hpool,
            tc.tile_pool(name="probs", bufs=4) as probs_pool,
            tc.tile_pool(name="vones", bufs=12) as vpool,
            tc.tile_pool(name="stage", bufs=4) as stage_pool,
            tc.tile_pool(name="psmm", bufs=4, space="PSUM") as ps_mm,
            tc.tile_pool(name="pspv", bufs=2, space="PSUM") as ps_pv,
            tc.tile_pool(name="psvt", bufs=2, space="PSUM") as ps_vt,
        ):
            ident = singles.tile([128, 128], F32)
            make_identity(nc, ident)

            w_sb = {}
            b_sb = {}
            for nm in ("q", "k", "v"):
                wt = singles.tile([128, KT, 128], F32R, tag=f"w{nm}")
                nc.sync.dma_start(
                    out=wt, in_=w_ap[nm].rearrange("(kt p) m -> p kt m", p=128)
                )
                w_sb[nm] = wt
                bt = singles.tile([128, 1], F32, tag=f"b{nm}")
                nc.sync.dma_start(out=bt, in_=b_ap[nm])
                b_sb[nm] = bt

            QT = big.tile([128, BS], F32R, tag="qt")
            vring = big.tile([128, 12, 130], F32R, tag="vring")
            nc.sync.dma_start(
                out=vring.rearrange("p s (x o) -> p s x o", x=2)[:, :, :, 64:65],
                in_=ones2.rearrange("p (s x o) -> p s x o", s=12, x=2, o=1),
            )
            KTt = big.tile([128, BS], F32R, tag="kt")
            VT = big.tile([128, BS], F32R, tag="vt")

            # ---- Phase 1: projections, output in [feature, seq] layout ----
            hsT_r = hsT.rearrange("(kt p) s -> p kt s", p=128)
            for n in range(NT):
                sl = slice(n * 512, (n + 1) * 512)
                hst0 = hpool.tile([128, 4, 512], F32R, tag="hst")
                hst1 = hpool.tile([128, 4, 512], F32R, tag="hst")
                nc.sync.dma_start(out=hst0, in_=hsT_r[:, 0:4, sl])
                nc.sync.dma_start(out=hst1, in_=hsT_r[:, 4:8, sl])
                halves = (hst0, hst1)
                for nm, dest, scale in (
                    ("q", QT, 1.0 / np.sqrt(D)),
                    ("k", KTt, 1.0),
                    ("v", VT, 1.0),
                ):
                    psp = ps_mm.tile([128, 512], F32, tag="mm")
                    for k in range(KT):
                        nc.tensor.matmul(
                            psp,
                            lhsT=w_sb[nm][:, k, :],
                            rhs=halves[k // 4][:, k % 4, :],
                            start=(k == 0),
                            stop=(k == KT - 1),
                        )
                    nc.scalar.activation(
                        dest[:, sl], psp, AFT.Identity, bias=b_sb[nm], scale=scale
                    )

            # ---- Phase 2: chunked attention, fully transposed ----
            vones = {}
            for b in range(B):
                base = b * S
                for c in range(NCHUNK):
                    lo = max(0, 2 * c - 2)
                    hi = min(NKB, 2 * c + 4)
                    n_kb = hi - lo

                    # V^T -> [keys, d] ring slots (+persistent ones col)
                    for kb in range(lo, hi):
                        if (b, kb) in vones:
                            continue
                        vt_ps = ps_vt.tile([128, 128], F32, tag="vt")
                        nc.tensor.transpose(
                            vt_ps,
                            VT[:, base + kb * 128 : base + (kb + 1) * 128].bitcast(F32),
                            ident,
                        )
                        slot = (2 * NKB * b + kb) % 12
                        nc.vector.tensor_copy(
                            vring[:, slot, :].rearrange("p (h x) -> p h x", h=2)[
                                :, :, 0:64
                            ],
                            vt_ps.rearrange("p (h x) -> p h x", h=2),
                        )
                        vones[(b, kb)] = slot

                    q_sl = slice(base + c * W, base + (c + 1) * W)
                    pr = {
                        h: probs_pool.tile(
                            [128, 6, 256], F32R, tag="probs", name=f"pr{h}_{b}_{c}"
                        )
                        for h in (0, 1)
                    }
                    for ip in range(n_kb // 2):
                        sps = {
                            h: ps_mm.tile(
                                [128, 2, 256], F32, tag="mm", name=f"s{h}_{b}_{c}_{ip}"
                            )
                            for h in (0, 1)
                        }
                        for j in (0, 1):
                            kb = lo + 2 * ip + j
                            k_sl = slice(base + kb * 128, base + (kb + 1) * 128)
                            for h in (0, 1):
                                d_sl = slice(h * 64, (h + 1) * 64)
                                nc.tensor.matmul(
                                    sps[h][:, j, :],
                                    lhsT=KTt[d_sl, k_sl],
                                    rhs=QT[d_sl, q_sl],
                                    start=True,
                                    stop=True,
                                )
                        for h in (0, 1):
                            nc.scalar.activation(
                                pr[h][:, 2 * ip : 2 * ip + 2, :], sps[h], AFT.Exp
                            )

                    o_sl_pre = slice(base + c * W, base + (c + 1) * W)
                    stage = stage_pool.tile([128, 256], F32, tag="stage")
                    for h in (0, 1):
                        po = ps_pv.tile([65, 256], F32, tag="pv")
                        for i in range(n_kb):
                            kb = lo + i
                            nc.tensor.matmul(
                                po,
                                lhsT=vring[:, vones[(b, kb)], h * 65 : (h + 1) * 65],
                                rhs=pr[h][:, i, :],
                                start=(i == 0),
                                stop=(i == n_kb - 1),
                            )
                        nc.vector.tensor_copy(stage[h * 64 : (h + 1) * 64, :], po[0:64, :])
                        dst_h = stage_pool.tile(
                            [1, 256], F32, tag=f"dstage{h}", name=f"dst{h}_{b}_{c}"
                        )
                        nc.vector.tensor_copy(dst_h, po[64:65, :])
                        nc.sync.dma_start(
                            out=outT[128 + h : 129 + h, o_sl_pre], in_=dst_h
                        )

                    nc.sync.dma_start(out=outT[0:128, o_sl_pre], in_=stage)

    nc.compile()
    return nc


def get_nc():
    global _NC_CACHE
    if _NC_CACHE is None:
        _NC_CACHE = _build()
    return _NC_CACHE


def make_in_maps(hidden_states, Wq, bq, Wk, bk, Wv, bv):
    hsT = np.ascontiguousarray(
        hidden_states.reshape(BS, E).T.astype(np.float32, copy=False)
    )
    ones2 = np.ones((128, 24), np.float32)
    in_maps = []
    for c in range(8):
        fsl = slice(c * 128, (c + 1) * 128)
        in_maps.append(
            {
                "hsT": hsT,
                "wqT": np.ascontiguousarray(Wq[fsl].T.astype(np.float32, copy=False)),
                "wkT": np.ascontiguousarray(Wk[fsl].T.astype(np.float32, copy=False)),
                "wvT": np.ascontiguousarray(Wv[fsl].T.astype(np.float32, copy=False)),
                "bq": np.ascontiguousarray(bq[fsl].reshape(128, 1) / np.sqrt(D)),
                "bk": np.ascontiguousarray(bk[fsl].reshape(128, 1)),
                "bv": np.ascontiguousarray(bv[fsl].reshape(128, 1)),
                "ones2": ones2,
            }
        )
    return in_maps


def assemble(results):
    """results: list of 8 per-core dicts with 'outT' [130, BS] -> full [B,S,E]."""
    # boundary pad mass: chunk 0 row ii has ii unmasked zero-score pad keys,
    # chunk 15 row ii has 255-ii
    pad = np.zeros(S, np.float32)
    pad[:W] = np.arange(W, dtype=np.float32)
    pad[S - W :] = (W - 1) - np.arange(W, dtype=np.float32)

    out = np.empty((B, S, E), np.float32)
    for c in range(8):
        oT = results[c]["outT"]  # [130, BS]
        num = oT[0:128].T.reshape(B, S, 2, 64)  # b, s, head_local, d
        den = oT[128:130].T.reshape(B, S, 2)  # b, s, head_local
        den = den + pad[None, :, None]
        out[:, :, c * 128 : (c + 1) * 128] = (num / den[..., None]).reshape(B, S, 128)
    return out


def kernel(hidden_states, Wq, bq, Wk, bk, Wv, bv):
    nc = get_nc()
    in_maps = make_in_maps(hidden_states, Wq, bq, Wk, bk, Wv, bv)
    res = run_bass_kernel_spmd(nc, in_maps, list(range(8)))
    return assemble(res.results)



# revision 3
# speedup vs baseline: 332.9022x; 332.9022x over previous
"""Longformer sliding-chunk attention (B=2, S=4096, E=1024, H=16, W=256) on 8 trn2 cores.

bf16 datapath, V projected directly in [seq, feat] layout (no PE
transposes), bias-adds on DVE, attention chunks interleaved with the
projection stream so PE/ACT/DMA overlap across the whole kernel, and
PSUM banked {1 projection, 6 scores, 1 pv} so a full chunk's six score
tiles are live at once (decouples consecutive chunks' QK->exp pipeline).

Sharding: tensor-parallel over heads — core c owns heads {2c, 2c+1}. Each core:
  - projects q/k for its 128 output features over the full [8192, 1024]
    hidden states into transposed [d, s] layout (bf16), and v into natural
    [s, d] layout with an interleaved ones column per head (for softmax
    denominators)
  - computes chunked attention fully transposed: scoresT = K @ Q^T per
    128-key-block (two heads concurrently on disjoint 64-row PE tiles),
    exp on ACT (no max subtraction: scores are O(1) for this problem),
    probsT @ V via PE; the ones column yields the denominators for free
  - ships unnormalized numerator^T [128, 8192] + denominators [2, 8192]
Host adds the boundary-mask pad mass to denominators and normalizes.

`repeat` wraps the whole per-core program in a hardware For_i loop that
re-executes it end-to-end — used by the timing harness to amortize
dispatch overhead; results are identical since the program is idempotent.
"""
import contextlib

import numpy as np

import concourse.bass as bass
import concourse.mybir as mybir
import concourse.tile as tile
from concourse import bacc
from concourse.bass_utils import run_bass_kernel_spmd

F32 = mybir.dt.float32
BF16 = mybir.dt.bfloat16
AFT = mybir.ActivationFunctionType

B, S, E = 2, 4096, 1024
H, W, D = 16, 256, 64
BS = B * S           # 8192
NT = 16              # 512-wide seq tiles over BS for projections
KT = 8               # contraction tiles of 128 over E
NCHUNK = S // W      # 16 chunks per batch
NKB = S // 128       # 32 key blocks of 128 per batch
NKBG = BS // 128     # 64 key blocks globally

_NC_CACHE = {}


def _build(repeat=1):
    nc = bacc.Bacc("TRN2", target_bir_lowering=False, debug=False, num_devices=8)

    hsT = nc.dram_tensor("hsT", [E, BS], BF16, kind="ExternalInput").ap()
    w_ap = {}
    for nm in ("q", "k", "v"):
        w_ap[nm] = nc.dram_tensor(f"w{nm}T", [E, 128], BF16, kind="ExternalInput").ap()
    bq_ap = nc.dram_tensor("bq", [128, 1], F32, kind="ExternalInput").ap()
    bk_ap = nc.dram_tensor("bk", [128, 1], F32, kind="ExternalInput").ap()
    bvr_ap = nc.dram_tensor("bvrow", [1, 128], BF16, kind="ExternalInput").ap()
    outT = nc.dram_tensor("outT", [130, BS], F32, kind="ExternalOutput").ap()

    with tile.TileContext(nc) as tc:
        with (
            tc.tile_pool(name="singles", bufs=1) as singles,
            tc.tile_pool(name="big", bufs=1) as big,
            tc.tile_pool(name="hst", bufs=4) as hpool,
            tc.tile_pool(name="probs", bufs=4) as probs_pool,
            tc.tile_pool(name="stage", bufs=4) as stage_pool,
            tc.tile_pool(name="psp1", bufs=2, space="PSUM") as ps_p1,
            tc.tile_pool(name="pssc", bufs=4, space="PSUM") as ps_sc,
            tc.tile_pool(name="pspv", bufs=2, space="PSUM") as ps_pv,
        ):
            w_sb = {}
            for nm in ("q", "k", "v"):
                wt = singles.tile([128, KT, 128], BF16, tag=f"w{nm}")
                nc.sync.dma_start(
                    out=wt, in_=w_ap[nm].rearrange("(kt p) m -> p kt m", p=128)
                )
                w_sb[nm] = wt
            bq_sb = singles.tile([128, 1], F32, tag="bq")
            nc.sync.dma_start(out=bq_sb, in_=bq_ap)
            bk_sb = singles.tile([128, 1], F32, tag="bk")
            nc.sync.dma_start(out=bk_sb, in_=bk_ap)
            bvr_sb = singles.tile([1, 128], BF16, tag="bvr")
            nc.sync.dma_start(out=bvr_sb, in_=bvr_ap)
            ones_sb = singles.tile([1, 128], BF16, tag="ones")
            nc.vector.memset(ones_sb, 1.0)

            QT = big.tile([128, BS], BF16, tag="qt")
            KTt = big.tile([128, BS], BF16, tag="kt")
            # V in natural [seq, feat] layout per 128-key block, with a ones
            # column appended per head: [:, kb, 0:64]=h0, 64=ones, 65:129=h1,
            # 129=ones
            VN = big.tile([128, NKBG, 130], BF16, tag="vn")
            nc.vector.memset(
                VN.rearrange("p s (h x) -> p s h x", h=2)[:, :, :, 64:65], 1.0
            )

            hsT_r = hsT.rearrange("(kt p) s -> p kt s", p=128)

            def emit_proj_tile(n):
                sl = slice(n * 512, (n + 1) * 512)
                hst0 = hpool.tile([128, 4, 512], BF16, tag="hst")
                hst1 = hpool.tile([128, 4, 512], BF16, tag="hst")
                nc.sync.dma_start(out=hst0, in_=hsT_r[:, 0:4, sl])
                nc.sync.dma_start(out=hst1, in_=hsT_r[:, 4:8, sl])
                halves = (hst0, hst1)
                # Q and K: [feat, seq] layout, N=512 matmuls
                for nm, dest, bias in (("q", QT, bq_sb), ("k", KTt, bk_sb)):
                    psp = ps_p1.tile([128, 512], F32, tag="p1")
                    for k in range(KT):
                        nc.tensor.matmul(
                            psp,
                            lhsT=w_sb[nm][:, k, :],
                            rhs=halves[k // 4][:, k % 4, :],
                            start=(k == 0),
                            stop=(k == KT - 1),
                        )
                    nc.vector.tensor_scalar_add(dest[:, sl], psp, bias)
                # V: natural [seq, feat] layout, N=128 matmuls + ones-row
                # matmul that accumulates the bias
                psv = ps_p1.tile([128, 512], F32, tag="p1")
                for sb in range(4):
                    o_sl = slice(sb * 128, (sb + 1) * 128)
                    for k in range(KT):
                        nc.tensor.matmul(
                            psv[:, o_sl],
                            lhsT=halves[k // 4][:, k % 4, o_sl],
                            rhs=w_sb["v"][:, k, :],
                            start=(k == 0),
                            stop=False,
                        )
                    nc.tensor.matmul(
                        psv[:, o_sl],
                        lhsT=ones_sb,
                        rhs=bvr_sb,
                        start=False,
                        stop=True,
                    )
                nc.vector.tensor_copy(
                    VN[:, 4 * n : 4 * n + 4, :].rearrange(
                        "p s (h x) -> p s h x", h=2
                    )[:, :, :, 0:64],
                    psv.rearrange("p (s h x) -> p s h x", s=4, h=2),
                )

            def emit_chunk(b, c):
                base = b * S
                lo = max(0, 2 * c - 2)
                hi = min(NKB, 2 * c + 4)
                n_kb = hi - lo
                q_sl = slice(base + c * W, base + (c + 1) * W)

                pr = {
                    h: probs_pool.tile(
                        [128, 6, 256], BF16, tag="probs", name=f"pr{h}_{b}_{c}"
                    )
                    for h in (0, 1)
                }
                for ip in range(n_kb // 2):
                    sps = {
                        h: ps_sc.tile(
                            [128, 2, 256], F32, tag="sc", name=f"s{h}_{b}_{c}_{ip}"
                        )
                        for h in (0, 1)
                    }
                    for j in (0, 1):
                        kb = lo + 2 * ip + j
                        k_sl = slice(base + kb * 128, base + (kb + 1) * 128)
                        for h in (0, 1):
                            d_sl = slice(h * 64, (h + 1) * 64)
                            nc.tensor.matmul(
                                sps[h][:, j, :],
                                lhsT=KTt[d_sl, k_sl],
                                rhs=QT[d_sl, q_sl],
                                start=True,
                                stop=True,
                                tile_position=(64 * h, 0),
                            )
                    for h in (0, 1):
                        nc.scalar.activation(
                            pr[h][:, 2 * ip : 2 * ip + 2, :], sps[h], AFT.Exp
                        )

                stage = stage_pool.tile([128, 256], F32, tag="stage")
                for h in (0, 1):
                    po = ps_pv.tile([65, 256], F32, tag="pv")
                    for i in range(n_kb):
                        kbg = b * NKB + lo + i
                        nc.tensor.matmul(
                            po,
                            lhsT=VN[:, kbg, h * 65 : (h + 1) * 65],
                            rhs=pr[h][:, i, :],
                            start=(i == 0),
                            stop=(i == n_kb - 1),
                        )
                    nc.vector.tensor_copy(stage[h * 64 : (h + 1) * 64, :], po[0:64, :])
                    dst_h = stage_pool.tile(
                        [1, 256], F32, tag=f"dstage{h}", name=f"dst{h}_{b}_{c}"
                    )
                    nc.vector.tensor_copy(dst_h, po[64:65, :])
                    nc.sync.dma_start(out=outT[128 + h : 129 + h, q_sl], in_=dst_h)
                nc.sync.dma_start(out=outT[0:128, q_sl], in_=stage)

            loop_cm = tc.For_i(0, repeat, 1) if repeat > 1 else contextlib.nullcontext()
            with loop_cm:
                for b in range(B):
                    for t in range(8):
                        emit_proj_tile(8 * b + t)
                        # chunks whose K/V window is fully projected
                        for c in ([0] if t == 0 else [2 * t - 1, 2 * t]):
                            emit_chunk(b, c)
                    emit_chunk(b, NCHUNK - 1)

    nc.compile()
    return nc


def get_nc(repeat=1):
    if repeat not in _NC_CACHE:
        _NC_CACHE[repeat] = _build(repeat)
    return _NC_CACHE[repeat]


def make_in_maps(hidden_states, Wq, bq, Wk, bk, Wv, bv):
    import ml_dtypes

    bf16 = ml_dtypes.bfloat16
    hsT = np.ascontiguousarray(
        hidden_states.reshape(BS, E).T.astype(bf16)
    )
    scale = 1.0 / np.sqrt(D)
    in_maps = []
    for c in range(8):
        fsl = slice(c * 128, (c + 1) * 128)
        in_maps.append(
            {
                "hsT": hsT,
                "wqT": np.ascontiguousarray((Wq[fsl] * scale).T.astype(bf16)),
                "wkT": np.ascontiguousarray(Wk[fsl].T.astype(bf16)),
                "wvT": np.ascontiguousarray(Wv[fsl].T.astype(bf16)),
                "bq": np.ascontiguousarray(
                    (bq[fsl] * scale).reshape(128, 1).astype(np.float32)
                ),
                "bk": np.ascontiguousarray(bk[fsl].reshape(128, 1).astype(np.float32)),
                "bvrow": np.ascontiguousarray(bv[fsl].reshape(1, 128).astype(bf16)),
            }
        )
    return in_maps


def assemble(results):
    """results: list of 8 per-core dicts with 'outT' [130, BS] -> full [B,S,E]."""
    # boundary pad mass: chunk 0 row ii has ii unmasked zero-score pad keys,
    # chunk 15 row ii has 255-ii
    pad = np.zeros(S, np.float32)
    pad[:W] = np.arange(W, dtype=np.float32)
    pad[S - W :] = (W - 1) - np.arange(W, dtype=np.float32)

    out = np.empty((B, S, E), np.float32)
    for c in range(8):
        oT = results[c]["outT"]  # [130, BS]
        num = oT[0:128].T.reshape(B, S, 2, 64)  # b, s, head_local, d
        den = oT[128:130].T.reshape(B, S, 2)  # b, s, head_local
        den = den + pad[None, :, None]
        out[:, :, c * 128 : (c + 1) * 128] = (num / den[..., None]).reshape(B, S, 128)
    return out


def kernel(hidden_states, Wq, bq, Wk, bk, Wv, bv):
    nc = get_nc()
    in_maps = make_in_maps(hidden_states, Wq, bq, Wk, bk, Wv, bv)
    res = run_bass_kernel_spmd(nc, in_maps, list(range(8)))
    return assemble(res.results)


# revision 4
# speedup vs baseline: 370.7888x; 1.1138x over previous
"""Longformer sliding-chunk attention (B=2, S=4096, E=1024, H=16, W=256) on 8 trn2 cores.

bf16 datapath; V projected directly in [seq, feat] layout (no PE
transposes); q/k bias-adds on DVE and the v bias folded into the host
assemble (out = (num + bv*den)/(den+pad)), so ACT does only exp;
attention chunks interleaved with the projection stream, each chunk's
PV emitted one chunk behind its QK/exp so PE runs the next QK while
ACT exps the current one; PSUM banked {1 projection, 6 scores, 1 pv}
so a full chunk's six score tiles are live at once.

Sharding: tensor-parallel over heads — core c owns heads {2c, 2c+1}. Each core:
  - projects q/k for its 128 output features over the full [8192, 1024]
    hidden states into transposed [d, s] layout (bf16), and v into natural
    [s, d] layout with an interleaved ones column per head (for softmax
    denominators)
  - computes chunked attention fully transposed: scoresT = K @ Q^T per
    128-key-block (two heads concurrently on disjoint 64-row PE tiles),
    exp on ACT (no max subtraction: scores are O(1) for this problem),
    probsT @ V via PE; the ones column yields the denominators for free
  - ships unnormalized numerator^T [128, 8192] + denominators [2, 8192]
Host adds the boundary-mask pad mass to denominators and normalizes.

`repeat` wraps the whole per-core program in a hardware For_i loop that
re-executes it end-to-end — used by the timing harness to amortize
dispatch overhead; results are identical since the program is idempotent.
"""
import contextlib

import numpy as np

import concourse.bass as bass
import concourse.mybir as mybir
import concourse.tile as tile
from concourse import bacc
from concourse.bass_utils import run_bass_kernel_spmd

F32 = mybir.dt.float32
BF16 = mybir.dt.bfloat16
AFT = mybir.ActivationFunctionType

B, S, E = 2, 4096, 1024
H, W, D = 16, 256, 64
BS = B * S           # 8192
NT = 16              # 512-wide seq tiles over BS for projections
KT = 8               # contraction tiles of 128 over E
NCHUNK = S // W      # 16 chunks per batch
NKB = S // 128       # 32 key blocks of 128 per batch
NKBG = BS // 128     # 64 key blocks globally

_NC_CACHE = {}
_LAST_BV = [None]  # stashed by make_in_maps for assemble's default


def _build(repeat=1):
    nc = bacc.Bacc("TRN2", target_bir_lowering=False, debug=False, num_devices=8)

    hsT = nc.dram_tensor("hsT", [E, BS], BF16, kind="ExternalInput").ap()
    w_ap = {}
    for nm in ("q", "k", "v"):
        w_ap[nm] = nc.dram_tensor(f"w{nm}T", [E, 128], BF16, kind="ExternalInput").ap()
    bq_ap = nc.dram_tensor("bq", [128, 1], F32, kind="ExternalInput").ap()
    bk_ap = nc.dram_tensor("bk", [128, 1], F32, kind="ExternalInput").ap()
    outT = nc.dram_tensor("outT", [130, BS], F32, kind="ExternalOutput").ap()

    with tile.TileContext(nc) as tc:
        with (
            tc.tile_pool(name="singles", bufs=1) as singles,
            tc.tile_pool(name="big", bufs=1) as big,
            tc.tile_pool(name="hst", bufs=4) as hpool,
            tc.tile_pool(name="probs", bufs=6) as probs_pool,
            tc.tile_pool(name="stage", bufs=4) as stage_pool,
            tc.tile_pool(name="psp1", bufs=2, space="PSUM") as ps_p1,
            tc.tile_pool(name="pssc", bufs=4, space="PSUM") as ps_sc,
            tc.tile_pool(name="pspv", bufs=2, space="PSUM") as ps_pv,
        ):
            w_sb = {}
            for nm in ("q", "k", "v"):
                wt = singles.tile([128, KT, 128], BF16, tag=f"w{nm}")
                nc.sync.dma_start(
                    out=wt, in_=w_ap[nm].rearrange("(kt p) m -> p kt m", p=128)
                )
                w_sb[nm] = wt
            bq_sb = singles.tile([128, 1], F32, tag="bq")
            nc.sync.dma_start(out=bq_sb, in_=bq_ap)
            bk_sb = singles.tile([128, 1], F32, tag="bk")
            nc.sync.dma_start(out=bk_sb, in_=bk_ap)

            QT = big.tile([128, BS], BF16, tag="qt")
            KTt = big.tile([128, BS], BF16, tag="kt")
            # V in natural [seq, feat] layout per 128-key block, with a ones
            # column appended per head: [:, kb, 0:64]=h0, 64=ones, 65:129=h1,
            # 129=ones
            VN = big.tile([128, NKBG, 130], BF16, tag="vn")
            nc.vector.memset(
                VN.rearrange("p s (h x) -> p s h x", h=2)[:, :, :, 64:65], 1.0
            )

            hsT_r = hsT.rearrange("(kt p) s -> p kt s", p=128)

            def emit_proj_tile(n):
                sl = slice(n * 512, (n + 1) * 512)
                hst0 = hpool.tile([128, 4, 512], BF16, tag="hst")
                hst1 = hpool.tile([128, 4, 512], BF16, tag="hst")
                nc.sync.dma_start(out=hst0[:, 0:2, :], in_=hsT_r[:, 0:2, sl])
                nc.sync.dma_start(out=hst0[:, 2:4, :], in_=hsT_r[:, 2:4, sl])
                nc.sync.dma_start(out=hst1[:, 0:2, :], in_=hsT_r[:, 4:6, sl])
                nc.sync.dma_start(out=hst1[:, 2:4, :], in_=hsT_r[:, 6:8, sl])
                halves = (hst0, hst1)
                # Q and K: [feat, seq] layout, N=512 matmuls
                for nm, dest, bias in (("q", QT, bq_sb), ("k", KTt, bk_sb)):
                    psp = ps_p1.tile([128, 512], F32, tag="p1")
                    for k in range(KT):
                        nc.tensor.matmul(
                            psp,
                            lhsT=w_sb[nm][:, k, :],
                            rhs=halves[k // 4][:, k % 4, :],
                            start=(k == 0),
                            stop=(k == KT - 1),
                        )
                    nc.vector.tensor_scalar_add(dest[:, sl], psp, bias)
                # V: natural [seq, feat] layout, N=128 matmuls + ones-row
                # matmul that accumulates the bias
                psv = ps_p1.tile([128, 512], F32, tag="p1")
                for sb in range(4):
                    o_sl = slice(sb * 128, (sb + 1) * 128)
                    for k in range(KT):
                        nc.tensor.matmul(
                            psv[:, o_sl],
                            lhsT=halves[k // 4][:, k % 4, o_sl],
                            rhs=w_sb["v"][:, k, :],
                            start=(k == 0),
                            stop=(k == KT - 1),
                        )
                nc.vector.tensor_copy(
                    VN[:, 4 * n : 4 * n + 4, :].rearrange(
                        "p s (h x) -> p s h x", h=2
                    )[:, :, :, 0:64],
                    psv.rearrange("p (s h x) -> p s h x", s=4, h=2),
                )

            def emit_qk(b, c):
                base = b * S
                lo = max(0, 2 * c - 2)
                hi = min(NKB, 2 * c + 4)
                n_kb = hi - lo
                q_sl = slice(base + c * W, base + (c + 1) * W)

                pr = {
                    h: probs_pool.tile(
                        [128, 6, 256], BF16, tag="probs", name=f"pr{h}_{b}_{c}"
                    )
                    for h in (0, 1)
                }
                for ip in range(n_kb // 2):
                    sps = {
                        h: ps_sc.tile(
                            [128, 2, 256], F32, tag="sc", name=f"s{h}_{b}_{c}_{ip}"
                        )
                        for h in (0, 1)
                    }
                    for j in (0, 1):
                        kb = lo + 2 * ip + j
                        k_sl = slice(base + kb * 128, base + (kb + 1) * 128)
                        for h in (0, 1):
                            d_sl = slice(h * 64, (h + 1) * 64)
                            nc.tensor.matmul(
                                sps[h][:, j, :],
                                lhsT=KTt[d_sl, k_sl],
                                rhs=QT[d_sl, q_sl],
                                start=True,
                                stop=True,
                                tile_position=(64 * h, 0),
                            )
                    for h in (0, 1):
                        nc.scalar.activation(
                            pr[h][:, 2 * ip : 2 * ip + 2, :], sps[h], AFT.Exp
                        )
                return pr

            def emit_pv(b, c, pr):
                base = b * S
                lo = max(0, 2 * c - 2)
                hi = min(NKB, 2 * c + 4)
                n_kb = hi - lo
                q_sl = slice(base + c * W, base + (c + 1) * W)

                stage = stage_pool.tile([128, 256], F32, tag="stage")
                for h in (0, 1):
                    po = ps_pv.tile([65, 256], F32, tag="pv")
                    for i in range(n_kb):
                        kbg = b * NKB + lo + i
                        nc.tensor.matmul(
                            po,
                            lhsT=VN[:, kbg, h * 65 : (h + 1) * 65],
                            rhs=pr[h][:, i, :],
                            start=(i == 0),
                            stop=(i == n_kb - 1),
                        )
                    nc.vector.tensor_copy(stage[h * 64 : (h + 1) * 64, :], po[0:64, :])
                    dst_h = stage_pool.tile(
                        [1, 256], F32, tag=f"dstage{h}", name=f"dst{h}_{b}_{c}"
                    )
                    nc.vector.tensor_copy(dst_h, po[64:65, :])
                    nc.sync.dma_start(out=outT[128 + h : 129 + h, q_sl], in_=dst_h)
                nc.sync.dma_start(out=outT[0:128, q_sl], in_=stage)

            loop_cm = tc.For_i(0, repeat, 1) if repeat > 1 else contextlib.nullcontext()
            with loop_cm:
                # PV trails QK/exp by one chunk so PE can run the next
                # chunk's QK while ACT is still exp'ing the current one
                fifo = []

                def push_chunk(b, c):
                    fifo.append((b, c, emit_qk(b, c)))
                    if len(fifo) > 1:
                        emit_pv(*fifo.pop(0))

                for b in range(B):
                    for t in range(8):
                        emit_proj_tile(8 * b + t)
                        # chunks whose K/V window is fully projected
                        for c in ([0] if t == 0 else [2 * t - 1, 2 * t]):
                            push_chunk(b, c)
                    push_chunk(b, NCHUNK - 1)
                while fifo:
                    emit_pv(*fifo.pop(0))

    nc.compile()
    return nc


def get_nc(repeat=1):
    if repeat not in _NC_CACHE:
        _NC_CACHE[repeat] = _build(repeat)
    return _NC_CACHE[repeat]


def make_in_maps(hidden_states, Wq, bq, Wk, bk, Wv, bv):
    import ml_dtypes

    bf16 = ml_dtypes.bfloat16
    _LAST_BV[0] = np.asarray(bv, np.float32)
    hsT = np.ascontiguousarray(
        hidden_states.reshape(BS, E).T.astype(bf16)
    )
    scale = 1.0 / np.sqrt(D)
    in_maps = []
    for c in range(8):
        fsl = slice(c * 128, (c + 1) * 128)
        in_maps.append(
            {
                "hsT": hsT,
                "wqT": np.ascontiguousarray((Wq[fsl] * scale).T.astype(bf16)),
                "wkT": np.ascontiguousarray(Wk[fsl].T.astype(bf16)),
                "wvT": np.ascontiguousarray(Wv[fsl].T.astype(bf16)),
                "bq": np.ascontiguousarray(
                    (bq[fsl] * scale).reshape(128, 1).astype(np.float32)
                ),
                "bk": np.ascontiguousarray(bk[fsl].reshape(128, 1).astype(np.float32)),
            }
        )
    return in_maps


def assemble(results, bv=None):
    """results: list of 8 per-core dicts with 'outT' [130, BS] -> full [B,S,E].

    The V bias is folded in here: v = v_raw + bv, and since pad keys carry
    v=0 with weight pad/(den+pad), out = (num + bv*den) / (den + pad)."""
    # boundary pad mass: chunk 0 row ii has ii unmasked zero-score pad keys,
    # chunk 15 row ii has 255-ii
    pad = np.zeros(S, np.float32)
    pad[:W] = np.arange(W, dtype=np.float32)
    pad[S - W :] = (W - 1) - np.arange(W, dtype=np.float32)

    if bv is None:
        bv = _LAST_BV[0] if _LAST_BV[0] is not None else np.zeros(E, np.float32)
    out = np.empty((B, S, E), np.float32)
    for c in range(8):
        oT = results[c]["outT"]  # [130, BS]
        num = oT[0:128].T.reshape(B, S, 2, 64)  # b, s, head_local, d
        den = oT[128:130].T.reshape(B, S, 2)  # b, s, head_local
        bvc = bv[c * 128 : (c + 1) * 128].reshape(2, 64).astype(np.float32)
        num = num + bvc[None, None] * den[..., None]
        den = den + pad[None, :, None]
        out[:, :, c * 128 : (c + 1) * 128] = (num / den[..., None]).reshape(B, S, 128)
    return out


def kernel(hidden_states, Wq, bq, Wk, bk, Wv, bv):
    nc = get_nc()
    in_maps = make_in_maps(hidden_states, Wq, bq, Wk, bk, Wv, bv)
    res = run_bass_kernel_spmd(nc, in_maps, list(range(8)))
    return assemble(res.results, bv=np.asarray(bv, np.float32))
